# revision 42
# baseline (speedup 1.0000x reference)
"""Causal self-attention (B=2, S=2048, E=2048, H=16) on 8 TRN2 NeuronCores.

Sharding: core = 4*b + hg  (b in {0,1} data-parallel over batch,
hg in {0..3} tensor-parallel over groups of 4 heads / Wqkv columns /
Wo rows).  Each core computes a partial [S, E] output (its heads'
contribution through Wo); the host sums the 4 partials per batch.

v2 design vs the fp32r baseline: all matmul operands are bf16 (same
1 cycle/row PE rate as fp32r>=256-wide, half the SBUF/DMA footprint).
That lets q/k/v stay SBUF-resident between projection and attention —
no DRAM spill round-trip.  Phase order: v-projection first (its e-outer
loop consumes xT tiles at the DMA stream rate), then q/k+RoPE per head,
then attention per head, then Wo.  fp32 is kept where it matters:
PSUM accumulation, softmax denominator accumulation, mask add, output.

Kernel math (per core, heads h=0..3 local):
  v [S, 512] natural layout (all 4 heads), e-loop outermost.
  qT/kT [D=128, S] = Wq/Wk_cols.T @ x[b].T   (xT host-transposed bf16)
  RoPE in [D, S] layout: rot = q*cos + swap(q)*sin*, where swap is a pure
  partition pair-swap done by two SBUF-to-SBUF DMAs (partition stride 2)
  and sin* carries the alternating sign; 1/sqrt(D) folded into Wq on host.
  scoresT [k, q] = kT_c.T @ qT_w  (D contraction, causal blocks only,
  diagonal blocks compute only columns >= ex0; host mask tiles).
  Phase 2 is software-pipelined in "units" (pairs of past blocks +
  single diagonal blocks): scores of unit u+1 are issued before the
  consumers of unit u, so ACT exp overlaps PE scores.  Past-block pairs
  share one 2-bank PSUM scores tile and a single batched exp (amortizes
  the ACT init); their softmax-denominator contribution is one
  ones-matmul on the DVE-computed pair-sum (halves denominator PE rows).
  oT [D, q] = sum_c v_c @ expP; oT *= recip(denom) (DVE).
  out_partial [S, E] = sum_h oT_h-slices @ Wo_rows (PE, h-outer so the
  lhsT is shared across the 4 window matmuls), DMA to DRAM fp32.
"""
import sys

if "/opt/trn_rl_repo" not in sys.path:
    sys.path.insert(0, "/opt/trn_rl_repo")

from contextlib import ExitStack

import numpy as np
import ml_dtypes

import concourse.bass as bass
import concourse.mybir as mybir
import concourse.tile as tile
from concourse import bacc, bass_utils

F32 = mybir.dt.float32
BF16 = mybir.dt.bfloat16
AF = mybir.ActivationFunctionType

B = 2
S = 2048
E = 2048
H = 16
D = 128
HL = 4            # heads per core
P = 128
NE = E // P       # 16 contraction chunks
NW = S // 512     # 4 q windows of 512
NSC = S // P      # 16 s-chunks
NEG = -1.0e30

_PROGRAM = None


def _build_program():
    nc = bacc.Bacc("TRN2", target_bir_lowering=False, debug=False, num_devices=8)

    xT = nc.dram_tensor("xT", [E, S], BF16, kind="ExternalInput").ap()
    wq = nc.dram_tensor("wq", [E, HL * D], BF16, kind="ExternalInput").ap()
    wk = nc.dram_tensor("wk", [E, HL * D], BF16, kind="ExternalInput").ap()
    wv = nc.dram_tensor("wv", [E, HL * D], BF16, kind="ExternalInput").ap()
    wo = nc.dram_tensor("wo", [HL * D, E], BF16, kind="ExternalInput").ap()
    cosf = nc.dram_tensor("cosf", [P, S], BF16, kind="ExternalInput").ap()
    sinf = nc.dram_tensor("sinf", [P, S], BF16, kind="ExternalInput").ap()
    onesc = nc.dram_tensor("onesc", [P, P], BF16, kind="ExternalInput").ap()
    tri = nc.dram_tensor("tri", [P, P], BF16, kind="ExternalInput").ap()
    outp = nc.dram_tensor("outp", [S, E], BF16, kind="ExternalOutput").ap()

    with tile.TileContext(nc) as tc:
        with ExitStack() as ctx_all:
            cpool = ctx_all.enter_context(tc.tile_pool(name="const", bufs=1))
            ones_t = cpool.tile([P, P], BF16, tag="ones")
            cos_t = cpool.tile([P, S], BF16, tag="cos")
            sin_t = cpool.tile([P, S], BF16, tag="sin")
            tri_t = cpool.tile([P, P], BF16, tag="tri")

            # persistent SBUF results of phase 1
            qkpool = ctx_all.enter_context(tc.tile_pool(name="qkT", bufs=2 * HL))
            qkT = [qkpool.tile([P, S], BF16, tag="qkT", name=f"qkT_{i}")
                   for i in range(2 * HL)]          # [q_h0, k_h0, q_h1, ...]
            vpool = ctx_all.enter_context(tc.tile_pool(name="v_sb", bufs=1))
            v_sb = vpool.tile([P, NSC, HL * D], BF16, tag="v")
            wo_pool = ctx_all.enter_context(tc.tile_pool(name="wo", bufs=HL))
            oT_pool = ctx_all.enter_context(tc.tile_pool(name="oT", bufs=HL * NW))

            # ---------------- Phase 1: QKV projection + RoPE ----------------
            with ExitStack() as ctx1:
                xpool = ctx1.enter_context(tc.tile_pool(name="xT", bufs=NE))
                wv_pool = ctx1.enter_context(tc.tile_pool(name="wv", bufs=NE))
                wqk_pool = ctx1.enter_context(tc.tile_pool(name="wqk", bufs=2))
                raw_pool = ctx1.enter_context(tc.tile_pool(name="raw", bufs=3))
                rsw_pool = ctx1.enter_context(tc.tile_pool(name="rsw", bufs=3))
                tmp_pool = ctx1.enter_context(tc.tile_pool(name="tmp", bufs=3))

                xt = []
                wvt = []
                for e in range(NE):
                    tw = wv_pool.tile([P, HL * D], BF16, tag="wv", name=f"wv_{e}")
                    nc.sync.dma_start(tw[:], wv[e * P:(e + 1) * P, :])
                    wvt.append(tw)
                    t = xpool.tile([P, S], BF16, tag="x", name=f"x_{e}")
                    nc.sync.dma_start(t[:], xT[e * P:(e + 1) * P, :])
                    xt.append(t)
                # constants and phase-2/3 weights are needed late; issue their
                # loads behind the xT stream so x_0 arrives as early as possible
                nc.sync.dma_start(cos_t[:], cosf)
                nc.sync.dma_start(sin_t[:], sinf)
                nc.sync.dma_start(ones_t[:], onesc)
                nc.sync.dma_start(tri_t[:], tri)
                wot = []
                for h2 in range(HL):
                    t = wo_pool.tile([P, E], BF16, tag="wo", name=f"wo_{h2}")
                    nc.sync.dma_start(t[:], wo[h2 * P:(h2 + 1) * P, :])
                    wot.append(t)

                # v first: e-outer loop consumes xT tiles at stream rate.
                # Group 1 runs bank-outer (tiles are all resident by then) so
                # each bank's psum->sbuf copy overlaps the later banks' matmuls
                # instead of bursting at the v->qk boundary.
                with ExitStack() as ctxv:
                    ps_v = ctxv.enter_context(tc.tile_pool(name="ps_v", bufs=8, space="PSUM"))
                    psvs = [ps_v.tile([P, 512], F32, tag="v", name=f"psv_0_{ci}") for ci in range(8)]
                    for e in range(NE):
                        for ci in range(8):
                            nc.tensor.matmul(psvs[ci][:], xt[e][:, ci * P:(ci + 1) * P],
                                             wvt[e][:],
                                             start=(e == 0), stop=(e == NE - 1))
                    for ci in range(8):
                        if ci % 2 == 0:
                            nc.scalar.activation(v_sb[:, ci, :], psvs[ci][:], AF.Copy)
                        else:
                            nc.vector.tensor_copy(v_sb[:, ci, :], psvs[ci][:])
                    psvs = [ps_v.tile([P, 512], F32, tag="v", name=f"psv_1_{ci}") for ci in range(8)]
                    for ci in range(8):
                        c = 8 + ci
                        for e in range(NE):
                            nc.tensor.matmul(psvs[ci][:], xt[e][:, c * P:(c + 1) * P],
                                             wvt[e][:],
                                             start=(e == 0), stop=(e == NE - 1))
                        if ci % 2 == 0:
                            nc.scalar.activation(v_sb[:, c, :], psvs[ci][:], AF.Copy)
                        else:
                            nc.vector.tensor_copy(v_sb[:, c, :], psvs[ci][:])

                # q, k with RoPE, per head
                ps_qk = ctx1.enter_context(tc.tile_pool(name="ps_qk", bufs=8, space="PSUM"))
                for h in range(HL):
                    for m, wsrc in ((0, wq), (1, wk)):
                        wt = wqk_pool.tile([P, NE, P], BF16, tag="wqk")
                        nc.sync.dma_start(
                            wt[:],
                            wsrc[:, h * D:(h + 1) * D].rearrange("(n p) c -> p n c", p=P))
                        pss4 = [ps_qk.tile([P, 512], F32, tag="qk",
                                           name=f"qkps_{h}_{m}_{w}") for w in range(NW)]
                        if h == HL - 1 and m == 1:
                            # last group: w-outer so each window's psum drains
                            # (RoPE chain) while later windows still matmul --
                            # otherwise attention's first psum tiles wait ~3us
                            # for this group's copies at the phase boundary
                            for w in range(NW):
                                for e in range(NE):
                                    nc.tensor.matmul(pss4[w][:], wt[:, e, :],
                                                     xt[e][:, w * 512:(w + 1) * 512],
                                                     start=(e == 0), stop=(e == NE - 1))
                        else:
                            # e-outer, w-inner: lhsT (wt[:, e, :]) shared across
                            # 4 matmuls
                            for e in range(NE):
                                for w in range(NW):
                                    nc.tensor.matmul(pss4[w][:], wt[:, e, :],
                                                     xt[e][:, w * 512:(w + 1) * 512],
                                                     start=(e == 0), stop=(e == NE - 1))
                        dst = qkT[2 * h + m]
                        for w in range(NW):
                            ws = slice(w * 512, (w + 1) * 512)
                            ps = pss4[w]
                            raw = raw_pool.tile([P, 512], BF16, tag="raw")
                            nc.scalar.activation(raw[:], ps[:], AF.Copy)
                            # pure pair-swap via two partition-strided
                            # SBUF-to-SBUF DMAs; the sign lives in sin*
                            rsw = rsw_pool.tile([P, 512], BF16, tag="rsw")
                            ar = raw[:].rearrange("(p two) f -> p two f", two=2)
                            rr = rsw[:].rearrange("(p two) f -> p two f", two=2)
                            nc.sync.dma_start(rr[:, 0, :], ar[:, 1, :])
                            nc.sync.dma_start(rr[:, 1, :], ar[:, 0, :])
                            tmp = tmp_pool.tile([P, 512], BF16, tag="tmp")
                            nc.vector.tensor_mul(tmp[:], raw[:], cos_t[:, ws])
                            nc.vector.tensor_mul(rsw[:], rsw[:], sin_t[:, ws])
                            nc.vector.tensor_add(dst[:, ws], tmp[:], rsw[:])

            # ---------------- Phase 2: attention per head ----------------
            with ExitStack() as ctx2:
                exp_pool = ctx2.enter_context(tc.tile_pool(name="expp", bufs=8))
                prs_pool = ctx2.enter_context(tc.tile_pool(name="prs", bufs=3))
                rec_pool = ctx2.enter_context(tc.tile_pool(name="rec", bufs=2))

                oTt = [[oT_pool.tile([P, 512], BF16, tag="oT", name=f"oT_{hh}_{w2}")
                        for w2 in range(NW)] for hh in range(HL)]

                with ExitStack() as ctx2p:
                    ps_s2 = ctx2p.enter_context(tc.tile_pool(name="ps_s2", bufs=2, space="PSUM"))
                    ps_o = ctx2p.enter_context(tc.tile_pool(name="ps_o", bufs=2, space="PSUM"))
                    ps_d = ctx2p.enter_context(tc.tile_pool(name="ps_d", bufs=2, space="PSUM"))

                    pending = []

                    def consume():
                        if not pending:
                            return
                        (kind, idx, h, w, pso, psd, ex2, start, stop) = pending.pop(0)
                        hs = slice(h * D, (h + 1) * D)
                        if kind == 0:          # pair of past blocks
                            j = idx
                            nc.tensor.matmul(pso[:], v_sb[:, j, hs], ex2[:, 0, :],
                                             start=start, stop=False)
                            nc.tensor.matmul(pso[:], v_sb[:, j + 1, hs], ex2[:, 1, :],
                                             start=False, stop=False)
                            prs = prs_pool.tile([P, 512], BF16, tag="prs")
                            nc.vector.tensor_add(prs[:], ex2[:, 0, :], ex2[:, 1, :])
                            nc.tensor.matmul(psd[:], ones_t[:], prs[:],
                                             start=start, stop=False)
                        else:                  # diagonal block r=idx
                            r = idx
                            c = 4 * w + r
                            ex0 = 128 * r
                            nc.tensor.matmul(pso[:, ex0:], v_sb[:, c, hs],
                                             ex2[:, 0, ex0:], start=start, stop=stop)
                            nc.tensor.matmul(psd[:, ex0:], ones_t[:], ex2[:, 0, ex0:],
                                             start=start, stop=stop)
                        if stop:
                            recd = rec_pool.tile([P, 512], F32, tag="r")
                            nc.vector.reciprocal_approx_fast(out=recd[:], in_=psd[:])
                            nc.vector.tensor_mul(oTt[h][w][:], pso[:], recd[:])

                    for h in range(HL):
                        qr = qkT[2 * h]
                        kr = qkT[2 * h + 1]
                        for w in range(NW):
                            ws = slice(w * 512, (w + 1) * 512)
                            pso = ps_o.tile([P, 512], F32, tag="o")
                            psd = ps_d.tile([P, 512], F32, tag="d")
                            units = [(0, j) for j in range(0, 4 * w, 2)]
                            units += [(1, r) for r in range(4)]
                            for i, (kind, idx) in enumerate(units):
                                if kind == 0:
                                    j = idx
                                    ps2 = ps_s2.tile([P, 2, 512], F32, tag="s2")
                                    nc.tensor.matmul(ps2[:, 0, :], kr[:, j * P:(j + 1) * P],
                                                     qr[:, ws], start=True, stop=True)
                                    nc.tensor.matmul(ps2[:, 1, :], kr[:, (j + 1) * P:(j + 2) * P],
                                                     qr[:, ws], start=True, stop=True)
                                    ex2 = exp_pool.tile([P, 2, 512], BF16, tag="e")
                                    nc.scalar.activation(ex2[:, :, :], ps2[:, :, :], AF.Exp)
                                else:
                                    r = idx
                                    c = 4 * w + r
                                    ex0 = 128 * r
                                    mms = slice(w * 512 + ex0, (w + 1) * 512)
                                    ps2 = ps_s2.tile([P, 2, 512], F32, tag="s2")
                                    nc.tensor.matmul(ps2[:, 0, ex0:], kr[:, c * P:(c + 1) * P],
                                                     qr[:, mms], start=True, stop=True)
                                    ex2 = exp_pool.tile([P, 2, 512], BF16, tag="e")
                                    nc.scalar.activation(ex2[:, 0, ex0:], ps2[:, 0, ex0:], AF.Exp)
                                    # causal mask applied post-exp: zero the
                                    # upper triangle of the diagonal 128-block
                                    # (cheap bf16 DVE mul, off the exp chain)
                                    nc.vector.tensor_mul(ex2[:, 0, ex0:ex0 + P],
                                                         ex2[:, 0, ex0:ex0 + P], tri_t[:])
                                if len(pending) >= 3:
                                    consume()
                                pending.append((kind, idx, h, w, pso, psd, ex2,
                                                i == 0, kind == 1 and idx == 3))
                    while pending:
                        consume()

                # ---------------- Phase 3: Wo ----------------
                with ExitStack() as ctx3p:
                    ps_out = ctx3p.enter_context(tc.tile_pool(name="ps_out", bufs=8, space="PSUM"))
                    ost_pool = ctx3p.enter_context(tc.tile_pool(name="ost", bufs=4))
                    for sc in range(NSC):
                        pos = [ps_out.tile([P, 512], F32, tag="po",
                                           name=f"po_{sc}_{w}") for w in range(NW)]
                        # h-outer: lhsT shared across the 4 window matmuls
                        for h2 in range(HL):
                            for w in range(NW):
                                nc.tensor.matmul(pos[w][:],
                                                 oTt[h2][sc // 4][:, (sc % 4) * P:(sc % 4 + 1) * P],
                                                 wot[h2][:, w * 512:(w + 1) * 512],
                                                 start=(h2 == 0), stop=(h2 == HL - 1))
                        for w in range(NW):
                            ws = slice(w * 512, (w + 1) * 512)
                            ost = ost_pool.tile([P, 512], BF16, tag="ost")
                            if w % 2 == 0:
                                nc.vector.tensor_copy(ost[:], pos[w][:])
                            else:
                                nc.scalar.activation(ost[:], pos[w][:], AF.Copy)
                            nc.sync.dma_start(outp[sc * P:(sc + 1) * P, ws], ost[:])

    nc.compile()
    return nc


def _get_program():
    global _PROGRAM
    if _PROGRAM is None:
        _PROGRAM = _build_program()
    return _PROGRAM


def _host_prep(x, Wqkv, Wo, freqs_cis):
    """Build the 8 per-core input maps."""
    bf = ml_dtypes.bfloat16
    x = np.asarray(x, dtype=np.float32)
    Wqkv = np.asarray(Wqkv, dtype=np.float32)
    Wo = np.asarray(Wo, dtype=np.float32)
    freqs_cis = np.asarray(freqs_cis, dtype=np.float32)

    scale = np.float32(D ** -0.5)
    cos = freqs_cis[:, 0, :, 0].T        # [64, S]
    sin = freqs_cis[:, 0, :, 1].T
    cosf = np.ascontiguousarray(np.repeat(cos, 2, axis=0)).astype(bf)
    # sin* with the pair-swap sign folded in: rot = q*cos + swap(q)*sin*
    # (swap is the pure partition pair-swap; row 2i needs -sin, row 2i+1 +sin)
    sinstar = np.repeat(sin, 2, axis=0)
    sinstar[0::2] *= -1.0
    sinf = np.ascontiguousarray(sinstar).astype(bf)

    onesc = np.ones((P, P), dtype=bf)

    # 0/1 causal mask for the diagonal 128-blocks, applied post-exp:
    # tri[k, q] = 1 if k <= q else 0 (same tile for every diagonal offset)
    kk = np.arange(P)[:, None]
    qq = np.arange(P)[None, :]
    tri = np.ascontiguousarray(np.where(kk <= qq, 1.0, 0.0)).astype(bf)

    in_maps = []
    for core in range(8):
        b, hg = divmod(core, 4)
        cs = slice(hg * 512, (hg + 1) * 512)
        in_maps.append({
            "xT": np.ascontiguousarray(x[b].T).astype(bf),
            "wq": (np.ascontiguousarray(Wqkv[:, 0 * E:1 * E][:, cs]) * scale).astype(bf),
            "wk": np.ascontiguousarray(Wqkv[:, 1 * E:2 * E][:, cs]).astype(bf),
            "wv": np.ascontiguousarray(Wqkv[:, 2 * E:3 * E][:, cs]).astype(bf),
            "wo": np.ascontiguousarray(Wo[hg * 512:(hg + 1) * 512, :]).astype(bf),
            "cosf": cosf,
            "sinf": sinf,
            "onesc": onesc,
            "tri": tri,
        })
    return in_maps


def run_cores(x, Wqkv, Wo, freqs_cis, trace=False, **kw):
    """Run the 8-core SPMD program; returns (partials list, BassKernelResults)."""
    nc = _get_program()
    in_maps = _host_prep(x, Wqkv, Wo, freqs_cis)
    res = bass_utils.run_bass_kernel_spmd(
        nc, in_maps, core_ids=list(range(8)), trace=trace, **kw)
    return [r["outp"] for r in res.results], res


def kernel(x, Wqkv, Wo, freqs_cis):
    partials, _ = run_cores(x, Wqkv, Wo, freqs_cis)
    out = np.empty((B, S, E), dtype=np.float32)
    for b in range(B):
        acc = partials[4 * b].astype(np.float32)
        for hg in range(1, 4):
            acc = acc + partials[4 * b + hg]
        out[b] = acc
    return out
